# revision 1
# baseline (speedup 1.0000x reference)
"""Trainium2 Bass kernel for nn_DFlashAttention_43774306681111.

Full-attention transformer block: QKV projection + per-head RMSNorm + neox
RoPE + GQA softmax attention (non-causal) + output projection.

Sharding (8 cores): 2-way data parallel over batch x 4-way tensor parallel
over heads. Core c handles batch c//4 and head group c%4 (q heads
4g..4g+3, kv head g). Each core computes a partial output [S, HID]
(its heads' contribution through Wo); the host sums the 4 partials per
batch. No device collectives.

Device layout: activations are kept transposed ([dim, token], dim on
partitions) so every matmul contracts on the partition axis:
  Q^T = Wq_tile^T @ X^T          (stationary Wq tile, moving X^T tile)
  S^T[k,q] = K^T_tile^T @ Q^T    (contraction d=128, one matmul per tile)
  softmax over k (= partitions): exp on ACT, sums via ones-vector matmul
  ctx^T[d,q] = V_tile^T @ expS^T (V stationary [k_tok, d])
  out[tok,hid] = ctxT_tile^T @ Wo
Matmuls run in float32r (fp32 storage, reduced-precision multiply);
PSUM accumulation is fp32. RoPE pairs (i, i+64) live on different
partitions, so the half-swap is done with two SBUF->SBUF DMAs and the
rotation sign is baked into the host-built sin table.

PE-stream discipline (the in-order PE queue is the bottleneck): every
matmul that depends on slow non-PE work is emitted at a point where that
work is already finished — eviction tails trickle out one per hid-tile
pair, V transposes ride the first attention block, the AV/sums stagger
carries across block boundaries, and Wo matmuls for a finished query
block are spread through the next block's QK stream.
"""
import numpy as np
from contextlib import ExitStack

import concourse.bass as bass
import concourse.tile as tile
from concourse import bacc, mybir
from concourse.bass_utils import run_bass_kernel_spmd

B, S, HID = 2, 2048, 2048
NH, NKV, D = 16, 4, 128
EPS = 1e-6
THETA = 1000000.0
SCALE = D ** -0.5

TP = 4                 # tensor-parallel groups (heads)
DP = 2                 # data-parallel over batch
HG = NH // TP          # q heads per core = 4
DQ = HG * D            # 512 q-proj cols per core
HALF = D // 2          # 64

F32 = mybir.dt.float32
F32R = mybir.dt.float32r
BF16 = mybir.dt.bfloat16

MM_DT = F32R           # attention/Wo matmul operand dtype (device-produced)
MM_NP = np.float32     # host dtype fed into MM_DT dram tensors
PROJ_DT = mybir.dt.float16   # projection operand dtype (halves phase A DMA)
PROJ_NP = np.float16

HT = HID // 128        # 16 hid tiles
TBS = 512              # token block size
NTB = S // TBS         # 4 token blocks
KT = S // 128          # 16 key tiles
QB = S // TBS          # 4 query blocks
NDT = HG + 2           # 6 projection outputs: q0..q3, k, v^T

STAGGER = 3            # AV matmul lag behind QK/exp (carried across blocks)

_cache = {}


def _build(skip_w=False):
    nc = bacc.Bacc(None, target_bir_lowering=False, debug=False)

    xt = nc.dram_tensor("xt", [HID, S], PROJ_DT, kind="ExternalInput")
    wq = nc.dram_tensor("wq", [HID, DQ], PROJ_DT, kind="ExternalInput")
    wk = nc.dram_tensor("wk", [HID, D], PROJ_DT, kind="ExternalInput")
    wv = nc.dram_tensor("wv", [HID, D], PROJ_DT, kind="ExternalInput")
    wo = nc.dram_tensor("wo", [DQ, HID], MM_DT, kind="ExternalInput")
    cos2 = nc.dram_tensor("cos2", [D, S], F32, kind="ExternalInput")
    sin2 = nc.dram_tensor("sin2", [D, S], F32, kind="ExternalInput")
    qnw = nc.dram_tensor("qnw", [D, 1], F32, kind="ExternalInput")
    knw = nc.dram_tensor("knw", [D, 1], F32, kind="ExternalInput")
    iden_d = nc.dram_tensor("iden", [128, 128], MM_DT, kind="ExternalInput")
    ones_d = nc.dram_tensor("ones", [128, 1], MM_DT, kind="ExternalInput")
    onesb_d = nc.dram_tensor("onesb", [128, 1], PROJ_DT, kind="ExternalInput")
    out = nc.dram_tensor("out", [S, HID], F32, kind="ExternalOutput")

    with tile.TileContext(nc) as tc, ExitStack() as ctx:
        const = ctx.enter_context(tc.tile_pool(name="const", bufs=1))
        big = ctx.enter_context(tc.tile_pool(name="big", bufs=1))
        blk = ctx.enter_context(tc.tile_pool(name="blk", bufs=8))
        outp = ctx.enter_context(tc.tile_pool(name="outp", bufs=3))
        scratch = ctx.enter_context(tc.tile_pool(name="scratch", bufs=2))
        rows = ctx.enter_context(tc.tile_pool(name="rows", bufs=2))
        psum = ctx.enter_context(tc.tile_pool(name="psum", bufs=1, space="PSUM"))

        # ---- constants ----
        ident = const.tile([128, 128], MM_DT)
        nc.scalar.dma_start(out=ident[:], in_=iden_d[:])
        ones_col = const.tile([128, 1], MM_DT)
        nc.scalar.dma_start(out=ones_col[:], in_=ones_d[:])
        onesb_col = const.tile([128, 1], PROJ_DT)
        nc.scalar.dma_start(out=onesb_col[:], in_=onesb_d[:])
        eps_row = const.tile([1, 1], F32)
        nc.vector.memset(eps_row, EPS)
        qnw_sb = const.tile([D, 1], F32)
        nc.scalar.dma_start(out=qnw_sb[:], in_=qnw[:])
        knw_sb = const.tile([D, 1], F32)
        nc.scalar.dma_start(out=knw_sb[:], in_=knw[:])

        # ---- resident weights / big activations (tag-shared slots) ----
        wq_sb = big.tile([128, HT, DQ], PROJ_DT, tag="bigw")
        wk_sb = big.tile([128, HT, D], PROJ_DT, tag="wk")
        wv_sb = big.tile([128, HT, D], PROJ_DT, tag="wv")
        cos_sb = big.tile([D, S], F32, tag="cos")
        sin_sb = big.tile([D, S], F32, tag="sin")

        qT = big.tile([D, HG, S], MM_DT, tag="qT")       # Q^T per head
        kT = big.tile([D, S], MM_DT, tag="kT")           # K^T
        vT = big.tile([D, S], MM_DT, tag="vT")           # V^T (pre-transpose)
        v_sb = big.tile([128, KT, D], MM_DT, tag="v")    # V [tok, d] tiles

        def stationary(ht, dt):
            if dt < HG:
                return wq_sb[:, ht, dt * D:(dt + 1) * D]
            if dt == HG:
                return wk_sb[:, ht, :]
            return wv_sb[:, ht, :]

        # Deferred eviction tails (rmsnorm + rope). Each contains one ssq
        # matmul; they are flushed ONE at a time at spread-out points of the
        # later PE stream so the single 'small' psum bank never backs up.
        pending_evict = []

        def flush_evict(k=1):
            for _ in range(min(k, len(pending_evict))):
                pending_evict.pop(0)()

        # ---- phase A: projections ----
        for tb in range(NTB):
            tsl = slice(tb * TBS, (tb + 1) * TBS)
            accs = [psum.tile([128, TBS], F32, tag=f"p{'ABCDEF'[dt]}",
                              name=f"acc_{tb}_{dt}") for dt in range(NDT)]
            for ht in range(HT):
                if tb == 0:
                    hsl = slice(ht * 128, (ht + 1) * 128)
                    nc.scalar.dma_start(out=wq_sb[:, ht, :], in_=wq[hsl, :])
                    nc.scalar.dma_start(out=wk_sb[:, ht, :], in_=wk[hsl, :])
                    nc.scalar.dma_start(out=wv_sb[:, ht, :], in_=wv[hsl, :])
                if ht == 1:
                    nc.scalar.dma_start(out=cos_sb[:, tsl], in_=cos2[:, tsl])
                    nc.scalar.dma_start(out=sin_sb[:, tsl], in_=sin2[:, tsl])
                if ht >= 2 and ht % 2 == 0:
                    flush_evict(1)  # previous tb's tails, one per ht pair
                xt_t = blk.tile([128, TBS], PROJ_DT, tag="xt", bufs=12, name=f"xt_{tb}_{ht}")
                nc.sync.dma_start(out=xt_t[:], in_=xt[ht * 128:(ht + 1) * 128, tsl])
                for dt in range(NDT):
                    nc.tensor.matmul(accs[dt][:], stationary(ht, dt), xt_t[:],
                                     start=(ht == 0), stop=(ht == HT - 1))
            for dt in [HG, NDT - 1, 0, 1, 2, 3]:
                acc = accs[dt]
                if dt == NDT - 1:
                    nc.scalar.copy(vT[:, tsl], acc[:])
                    continue
                w_ap = qnw_sb if dt < HG else knw_sb
                # single psum read (alternating engines) frees the bank fast
                raw = scratch.tile([128, TBS], F32, tag="raw", bufs=6,
                                   name=f"raw_{tb}_{dt}")
                if dt % 2 == 0:
                    nc.vector.tensor_copy(raw[:], acc[:])
                else:
                    nc.scalar.copy(raw[:], acc[:])
                # qn/q2 computed eagerly so the deferred ssq matmul is
                # ready the moment it lands in the PE stream
                if skip_w:
                    qn = raw  # norm weights are all-ones: skip the multiply
                else:
                    qn = scratch.tile([128, TBS], F32, tag="qn", bufs=6,
                                      name=f"qn_{tb}_{dt}")
                    nc.scalar.activation(qn[:], raw[:],
                                         mybir.ActivationFunctionType.Copy,
                                         scale=w_ap[:])
                q2 = scratch.tile([128, TBS], PROJ_DT, tag="q2", bufs=6,
                                  name=f"q2_{tb}_{dt}")
                nc.vector.tensor_mul(q2[:], raw[:], raw[:])

                def evict_tail(tb=tb, dt=dt, qn=qn, q2=q2, tsl=tsl):
                    ssq = psum.tile([1, TBS], F32, tag="small", bufs=1,
                                    name=f"ssq_{tb}_{dt}")
                    nc.tensor.matmul(ssq[:], onesb_col[:], q2[:],
                                     start=True, stop=True)
                    rstd = rows.tile([1, TBS], F32, tag="rstd", bufs=2,
                                     name=f"rstd_{tb}_{dt}")
                    nc.scalar.activation(rstd[:], ssq[:],
                                         mybir.ActivationFunctionType.Sqrt,
                                         scale=1.0 / D, bias=eps_row[:])
                    nc.vector.reciprocal_approx_fast(out=rstd[:], in_=rstd[:])
                    rstdb = scratch.tile([128, TBS], F32, tag="bcast", bufs=3,
                                         name=f"rstdb_{tb}_{dt}")
                    nc.gpsimd.partition_broadcast(rstdb[:], rstd[:])
                    # rope: swap halves via SBUF->SBUF DMA; sign baked in sin2
                    xsw = scratch.tile([128, TBS], F32, tag="xsw", bufs=2,
                                       name=f"xsw_{tb}_{dt}")
                    nc.sync.dma_start(out=xsw[0:HALF, :], in_=qn[HALF:D, :])
                    nc.sync.dma_start(out=xsw[HALF:D, :], in_=qn[0:HALF, :])
                    tmp = scratch.tile([128, TBS], F32, tag="tmp", bufs=2,
                                       name=f"tmp_{tb}_{dt}")
                    nc.vector.tensor_mul(tmp[:], qn[:], cos_sb[:, tsl])
                    sv = scratch.tile([128, TBS], F32, tag="sv", bufs=2,
                                      name=f"sv_{tb}_{dt}")
                    nc.gpsimd.tensor_mul(sv[:], xsw[:], sin_sb[:, tsl])
                    qro = scratch.tile([128, TBS], F32, tag="qro", bufs=2,
                                       name=f"qro_{tb}_{dt}")
                    nc.vector.tensor_add(qro[:], tmp[:], sv[:])
                    dest = qT[:, dt, tsl] if dt < HG else kT[:, tsl]
                    nc.vector.tensor_mul(dest, qro[:], rstdb[:])
                pending_evict.append(evict_tail)

        # ctx^T per head; slots reuse cos/sin/vT space (dead by phase B)
        ctxT = [big.tile([D, S], MM_DT, tag=t, name=f"ctxT_{h}")
                for h, t in enumerate(["cos", "sin", "vT", "ctx3"])]

        # wo loads overlap the first attention blocks ("bigw" frees after
        # the last projection matmul)
        wo_sb = big.tile([128, HG, HID], MM_DT, tag="bigw")
        for ct in range(HG):
            nc.scalar.dma_start(out=wo_sb[:, ct, :],
                              in_=wo[ct * 128:(ct + 1) * 128, :])

        # V transposes ride the first attention block's PE stream
        pending_tp = list(range(KT))

        def flush_tp(k=1):
            for _ in range(min(k, len(pending_tp))):
                kt0 = pending_tp.pop(0)
                tp = psum.tile([128, 128], MM_DT, tag="pE", name=f"tp_{kt0}")
                nc.tensor.transpose(tp[:], vT[:, kt0 * 128:(kt0 + 1) * 128],
                                    ident[:])
                if kt0 % 2 == 0:
                    nc.vector.tensor_copy(v_sb[:, kt0, :], tp[:])
                else:
                    nc.scalar.copy(v_sb[:, kt0, :], tp[:])

        # ---- phase B: attention (qb-major) with Wo folded in ----
        pending_wo = []

        def emit_wo(qb):
            thunks = []
            for tt in range(qb * (TBS // 128), (qb + 1) * (TBS // 128)):
                for hc in range(HID // TBS):
                    def thunk(tt=tt, hc=hc):
                        o_ps = psum.tile([128, TBS], F32,
                                         tag=f"p{'EF'[(tt * 4 + hc) % 2]}",
                                         name=f"o_{tt}_{hc}")
                        for ct in range(HG):
                            nc.tensor.matmul(
                                o_ps[:],
                                ctxT[ct][:, tt * 128:(tt + 1) * 128],
                                wo_sb[:, ct, hc * TBS:(hc + 1) * TBS],
                                start=(ct == 0), stop=(ct == HG - 1))
                        o_sb = outp.tile([128, TBS], F32, tag="osb",
                                         name=f"osb_{tt}_{hc}")
                        nc.scalar.copy(o_sb[:], o_ps[:])
                        nc.sync.dma_start(
                            out=out[tt * 128:(tt + 1) * 128,
                                    hc * TBS:(hc + 1) * TBS],
                            in_=o_sb[:])
                    thunks.append(thunk)
            return thunks

        def flush_wo(k):
            for _ in range(min(k, len(pending_wo))):
                pending_wo.pop(0)()

        # cross-block AV/sums stagger; entries: (kt, e, ctx_ps, sum_ps, blk)
        pend = []
        norm_jobs = {}

        def flush_av():
            kt0, e0, c_ps, s2_ps, bi = pend.pop(0)
            nc.tensor.matmul(c_ps[:], v_sb[:, kt0, :], e0[:],
                             start=(kt0 == 0), stop=(kt0 == KT - 1))
            nc.tensor.matmul(s2_ps[:], ones_col[:], e0[:],
                             start=(kt0 == 0), stop=(kt0 == KT - 1))
            if kt0 == KT - 1 and bi in norm_jobs:
                norm_jobs.pop(bi)()

        for qb in range(QB):
            qsl = slice(qb * TBS, (qb + 1) * TBS)
            for h in range(HG):
                blk_i = qb * HG + h
                ctx_ps = psum.tile([128, TBS], F32,
                                   tag=f"p{'CD'[blk_i % 2]}",
                                   name=f"ctx_{h}_{qb}")
                sum_ps = psum.tile([1, TBS], F32,
                                   tag=["small", "pG"][blk_i % 2], bufs=1,
                                   name=f"sum_{h}_{qb}")

                for kt in range(KT):
                    g = blk_i * KT + kt
                    s_ps = psum.tile([128, TBS], F32,
                                     tag=f"p{'AB'[g % 2]}",
                                     name=f"s_{h}_{qb}_{kt}")
                    nc.tensor.matmul(s_ps[:], kT[:, kt * 128:(kt + 1) * 128],
                                     qT[:, h, qsl], start=True, stop=True)
                    e = blk.tile([128, TBS], MM_DT, tag="blk",
                                 name=f"e_{h}_{qb}_{kt}")
                    nc.scalar.activation(e[:], s_ps[:],
                                         mybir.ActivationFunctionType.Exp,
                                         scale=SCALE)
                    pend.append((kt, e, ctx_ps, sum_ps, blk_i))
                    if len(pend) > STAGGER:
                        flush_av()
                    # spread deferred work through the first block's stream
                    if blk_i == 0:
                        if kt % 2 == 0:
                            flush_evict(1)
                        if kt >= 2:
                            flush_tp(2)
                    if h == 0 and qb > 0 and kt in (4, 6, 8, 10):
                        flush_wo(4)

                def norm_job(h=h, qb=qb, qsl=qsl, ctx_ps=ctx_ps,
                             sum_ps=sum_ps):
                    recip = rows.tile([1, TBS], F32, tag="recip",
                                      name=f"recip_{h}_{qb}")
                    nc.vector.reciprocal_approx_fast(out=recip[:], in_=sum_ps[:])
                    recipb = scratch.tile([128, TBS], F32, tag="bcast",
                                          bufs=3, name=f"recipb_{h}_{qb}")
                    nc.gpsimd.partition_broadcast(recipb[:], recip[:])
                    nc.vector.tensor_mul(ctxT[h][:, qsl], ctx_ps[:], recipb[:])
                norm_jobs[blk_i] = norm_job
            pending_wo.extend(emit_wo(qb))

        while pend:
            flush_av()
        for i in sorted(list(norm_jobs)):
            norm_jobs.pop(i)()
        flush_wo(len(pending_wo))

    nc.compile()
    return nc


def _prep_inputs(hidden_states, positions, Wq, Wk, Wv, Wo, q_norm_w, k_norm_w):
    hidden_states = np.asarray(hidden_states, dtype=np.float32)
    positions = np.asarray(positions)
    Wq = np.asarray(Wq, dtype=np.float32)
    Wk = np.asarray(Wk, dtype=np.float32)
    Wv = np.asarray(Wv, dtype=np.float32)
    Wo = np.asarray(Wo, dtype=np.float32)
    q_norm_w = np.asarray(q_norm_w, dtype=np.float32)
    k_norm_w = np.asarray(k_norm_w, dtype=np.float32)

    import ml_dtypes
    inv_freq = THETA ** (-np.arange(HALF, dtype=np.float32) / HALF)
    in_maps = []
    for c in range(DP * TP):
        b, g = divmod(c, TP)
        freqs = positions[b].astype(np.float32)[:, None] * inv_freq[None, :]  # [S,64]
        cos = np.cos(freqs).T.astype(np.float32)      # [64, S]
        sin = np.sin(freqs).T.astype(np.float32)
        cos2 = np.ascontiguousarray(np.concatenate([cos, cos], axis=0))   # [128,S]
        sin2 = np.ascontiguousarray(np.concatenate([-sin, sin], axis=0))  # [128,S]
        in_maps.append({
            "xt": np.ascontiguousarray(hidden_states[b].T).astype(PROJ_NP),
            "wq": np.ascontiguousarray(Wq[:, g * DQ:(g + 1) * DQ]).astype(PROJ_NP),
            "wk": np.ascontiguousarray(Wk[:, g * D:(g + 1) * D]).astype(PROJ_NP),
            "wv": np.ascontiguousarray(Wv[:, g * D:(g + 1) * D]).astype(PROJ_NP),
            "wo": np.ascontiguousarray(Wo[g * DQ:(g + 1) * DQ, :]).astype(MM_NP),
            "cos2": cos2,
            "sin2": sin2,
            "qnw": np.ascontiguousarray(q_norm_w[:, None]),
            "knw": np.ascontiguousarray(k_norm_w[:, None]),
            "iden": np.eye(128, dtype=MM_NP),
            "ones": np.ones((128, 1), dtype=MM_NP),
            "onesb": np.ones((128, 1), dtype=np.float16),
        })
    return in_maps


def _run(inputs, trace=False):
    skip_w = bool(np.allclose(inputs["q_norm_w"], 1.0)
                  and np.allclose(inputs["k_norm_w"], 1.0))
    key = ("nc", skip_w)
    if key not in _cache:
        _cache[key] = _build(skip_w)
    nc = _cache[key]
    in_maps = _prep_inputs(**inputs)
    res = run_bass_kernel_spmd(nc, in_maps, core_ids=list(range(DP * TP)),
                               trace=trace)
    out = np.zeros((B, S, HID), dtype=np.float32)
    for c in range(DP * TP):
        out[c // TP] += res.results[c]["out"]
    return out, res


def kernel(**inputs):
    out, _ = _run(inputs, trace=False)
    return out



# revision 5
# speedup vs baseline: 1.3620x; 1.3620x over previous
"""Trainium2 Bass kernel for nn_DFlashAttention_43774306681111.

Full-attention transformer block: QKV projection + per-head RMSNorm + neox
RoPE + GQA softmax attention (non-causal) + output projection.

Sharding (8 cores): 2-way data parallel over batch x 4-way tensor parallel
over heads. Core c handles batch c//4 and head group g=c%4 (q heads
4g..4g+3, kv head g). Each core computes a partial output [S, HID]
(its heads' contribution through Wo); the host sums the 4 partials per
batch. No device collectives.

Device layout: activations kept transposed ([dim, token], dim on
partitions) so every matmul contracts on the partition axis:
  Q^T = Wq_tile^T @ X^T          (stationary Wq tile, moving X^T tile)
  S^T[k,q] = K^T_tile^T @ Q^T    (contraction d=128, one matmul per tile)
  softmax over k (= partitions): exp on ACT -> bf16; the denominator is
    accumulated with bf16 DVE adds (eacc += e_kt) and reduced with ONE
    ones-vector matmul per block (instead of one per key tile).
  ctx^T[d,q] = V_tile^T @ expS^T (V stationary [k_tok, d])
  out[tok,hid] = ctxT_tile^T @ Wo

RoPE: the head dims are PERMUTED host-side (Wq/Wk columns, cos/sin
tables, norm weights) so the rotation pair (i, i+64) sits on adjacent
partitions (2i, 2i+1). The half-swap is then a single DVE stream_shuffle
(even<->odd within each 32-partition quadrant) instead of two SBUF->SBUF
DMAs. Scores/outputs are unchanged because QK contracts over the (same)
permutation of both q and k, and v/ctx/Wo are untouched.

Engine discipline: ACT runs ONLY exp in the attention phase (and one
batched 5-row sqrt per token block in phase A) so its activation table
never thrashes. All PSUM evacuations go to DVE/GPSIMD. RMSNorm
sqrt/recip are batched 5 tails at a time ([5,512] ops). Wo matmuls are
spread one output tile (4 matmuls) at a time through the next q-block's
PE stream so exp never starves behind a long Wo burst.
"""
import numpy as np
from contextlib import ExitStack

import concourse.bass as bass
import concourse.tile as tile
from concourse import bacc, mybir
from concourse.bass_utils import run_bass_kernel_spmd

B, S, HID = 2, 2048, 2048
NH, NKV, D = 16, 4, 128
EPS = 1e-6
THETA = 1000000.0
SCALE = D ** -0.5

TP = 4                 # tensor-parallel groups (heads)
DP = 2                 # data-parallel over batch
HG = NH // TP          # q heads per core = 4
DQ = HG * D            # 512 q-proj cols per core
HALF = D // 2          # 64

F32 = mybir.dt.float32
F32R = mybir.dt.float32r
BF16 = mybir.dt.bfloat16
FP16 = mybir.dt.float16

MM_DT = F32R           # v / ctx / Wo matmul operand dtype
MM_NP = np.float32
PROJ_DT = FP16         # projection operand dtype (halves phase A DMA)
PROJ_NP = np.float16

HT = HID // 128        # 16 hid tiles
TBS = 512              # token block size
NTB = S // TBS         # 4 token blocks
KT = S // 128          # 16 key tiles
QB = S // TBS          # 4 query blocks
NDT = HG + 2           # 6 projection outputs: q0..q3, k, v^T

STAGGER = 4            # AV matmul lag behind QK/exp

SWAP_MASK = [i ^ 1 for i in range(32)]   # even<->odd partition swap

_cache = {}


def _build(skip_w=False):
    nc = bacc.Bacc(None, target_bir_lowering=False, debug=False)

    xt = nc.dram_tensor("xt", [HID, S], PROJ_DT, kind="ExternalInput")
    wq = nc.dram_tensor("wq", [HID, DQ], PROJ_DT, kind="ExternalInput")
    wk = nc.dram_tensor("wk", [HID, D], PROJ_DT, kind="ExternalInput")
    wv = nc.dram_tensor("wv", [HID, D], PROJ_DT, kind="ExternalInput")
    wo = nc.dram_tensor("wo", [DQ, HID], MM_DT, kind="ExternalInput")
    cos2 = nc.dram_tensor("cos2", [D, S], F32, kind="ExternalInput")
    sin2 = nc.dram_tensor("sin2", [D, S], F32, kind="ExternalInput")
    qnw = nc.dram_tensor("qnw", [D, 1], F32, kind="ExternalInput")
    knw = nc.dram_tensor("knw", [D, 1], F32, kind="ExternalInput")
    iden_d = nc.dram_tensor("iden", [128, 128], BF16, kind="ExternalInput")
    onesb_d = nc.dram_tensor("onesb", [128, 1], BF16, kind="ExternalInput")
    out = nc.dram_tensor("out", [S, HID], F32, kind="ExternalOutput")

    with tile.TileContext(nc) as tc, ExitStack() as ctx:
        const = ctx.enter_context(tc.tile_pool(name="const", bufs=1))
        big = ctx.enter_context(tc.tile_pool(name="big", bufs=1))
        blk = ctx.enter_context(tc.tile_pool(name="blk", bufs=8))
        outp = ctx.enter_context(tc.tile_pool(name="outp", bufs=3))
        scratch = ctx.enter_context(tc.tile_pool(name="scratch", bufs=2))
        rows = ctx.enter_context(tc.tile_pool(name="rows", bufs=2))
        psum = ctx.enter_context(tc.tile_pool(name="psum", bufs=1, space="PSUM"))

        # ---- constants ----
        ident = const.tile([128, 128], BF16)
        nc.scalar.dma_start(out=ident[:], in_=iden_d[:])
        onesb_col = const.tile([128, 1], BF16)
        nc.scalar.dma_start(out=onesb_col[:], in_=onesb_d[:])
        eps1 = const.tile([1, 1], F32)
        nc.vector.memset(eps1, EPS)
        qnw_sb = const.tile([D, 1], F32)
        nc.scalar.dma_start(out=qnw_sb[:], in_=qnw[:])
        knw_sb = const.tile([D, 1], F32)
        nc.scalar.dma_start(out=knw_sb[:], in_=knw[:])

        # ---- resident weights / big activations (tag-shared slots) ----
        wq_sb = big.tile([128, HT, DQ], PROJ_DT, tag="bigw")
        wk_sb = big.tile([128, HT, D], PROJ_DT, tag="wk")
        wv_sb = big.tile([128, HT, D], PROJ_DT, tag="wv")
        cos_sb = big.tile([D, S], F32, tag="cos")
        sin_sb = big.tile([D, S], F32, tag="sin")

        qT = big.tile([D, HG, S], BF16, tag="qT")        # Q^T per head
        kT = big.tile([D, S], BF16, tag="kT")            # K^T
        vT = big.tile([D, S], BF16, tag="vT")            # V^T (pre-transpose)
        v_sb = big.tile([128, KT, D], BF16, tag="v")     # V [tok, d] tiles

        def stationary(ht, dt):
            if dt < HG:
                return wq_sb[:, ht, dt * D:(dt + 1) * D]
            if dt == HG:
                return wk_sb[:, ht, :]
            return wv_sb[:, ht, :]

        # Deferred PE work from the rmsnorm tails: one ssq matmul + its
        # psum->sbuf copy per projection output. Flushed one at a time at
        # spread-out points of the later PE stream.
        pending_pe = []

        def flush_pe(k=1):
            for _ in range(min(k, len(pending_pe))):
                pending_pe.pop(0)()

        # ---- phase A: projections ----
        # per-tb tail state, built at the end of each tb
        for tb in range(NTB):
            tsl = slice(tb * TBS, (tb + 1) * TBS)
            accs = [psum.tile([128, TBS], F32, tag=f"p{'ABCDEF'[dt]}",
                              name=f"acc_{tb}_{dt}") for dt in range(NDT)]
            for ht in range(HT):
                if tb == 0:
                    hsl = slice(ht * 128, (ht + 1) * 128)
                    nc.scalar.dma_start(out=wq_sb[:, ht, :], in_=wq[hsl, :])
                    nc.scalar.dma_start(out=wk_sb[:, ht, :], in_=wk[hsl, :])
                    nc.scalar.dma_start(out=wv_sb[:, ht, :], in_=wv[hsl, :])
                if ht == 1:
                    nc.scalar.dma_start(out=cos_sb[:, tsl], in_=cos2[:, tsl])
                    nc.scalar.dma_start(out=sin_sb[:, tsl], in_=sin2[:, tsl])
                if ht in (4, 6, 8, 10, 12, 14):
                    flush_pe(1)  # previous tb's ssq matmuls + tail chain
                xt_t = blk.tile([128, TBS], PROJ_DT, tag="xt", bufs=12,
                                name=f"xt_{tb}_{ht}")
                nc.sync.dma_start(out=xt_t[:], in_=xt[ht * 128:(ht + 1) * 128, tsl])
                for dt in range(NDT):
                    nc.tensor.matmul(accs[dt][:], stationary(ht, dt), xt_t[:],
                                     start=(ht == 0), stop=(ht == HT - 1))

            # ---- end of tb: evacuate psums fast, then queue tail work ----
            raws = {}
            q2s = {}
            for i, dt in enumerate([HG, NDT - 1, 0, 1, 2, 3]):
                acc = accs[dt]
                if dt == NDT - 1:
                    # v^T: straight evac in matmul dtype
                    nc.vector.tensor_copy(vT[:, tsl], acc[:])
                    continue
                raw = scratch.tile([128, TBS], BF16, tag="raw", bufs=10,
                                   name=f"raw_{tb}_{dt}")
                nc.vector.tensor_copy(raw[:], acc[:])
                q2 = scratch.tile([128, TBS], BF16, tag="q2", bufs=10,
                                  name=f"q2_{tb}_{dt}")
                if i % 2 == 0:
                    nc.gpsimd.tensor_mul(q2[:], raw[:], raw[:])
                else:
                    nc.vector.tensor_mul(q2[:], raw[:], raw[:])
                raws[dt] = raw
                q2s[dt] = q2

            # per-tail thunks: ssq matmul (PE) + sqrt direct from PSUM +
            # recip + broadcast + stream-shuffle rope. Deferred into the
            # next tb's PE stream (k first: needed earliest in attention).
            tail_dts = [HG, 0, 1, 2, 3]

            def make_tail(i, dt, tb=tb, tsl=tsl, raws=raws, q2s=q2s):
                def emit():
                    ssq = psum.tile([1, TBS], F32, tag=["pG", "pH"][i % 2],
                                    bufs=1, name=f"ssq_{tb}_{dt}")
                    nc.tensor.matmul(ssq[:], onesb_col[:], q2s[dt][:],
                                     start=True, stop=True)
                    rst = rows.tile([1, TBS], F32, tag="rst", bufs=4,
                                    name=f"rst_{tb}_{dt}")
                    nc.scalar.activation(rst[:], ssq[:],
                                         mybir.ActivationFunctionType.Sqrt,
                                         scale=1.0 / D, bias=eps1[:])
                    rstr = rows.tile([1, TBS], F32, tag="rstr", bufs=4,
                                     name=f"rstr_{tb}_{dt}")
                    nc.vector.reciprocal_approx_fast(out=rstr[:], in_=rst[:])
                    raw = raws[dt]
                    rstdb = scratch.tile([128, TBS], F32, tag="bcast",
                                         bufs=6, name=f"rstdb_{tb}_{dt}")
                    nc.gpsimd.partition_broadcast(rstdb[:], rstr[:])
                    if not skip_w:
                        w_ap = qnw_sb if dt < HG else knw_sb
                        nc.gpsimd.tensor_scalar_mul(rstdb[:], rstdb[:],
                                                    w_ap[:])
                    xsw = scratch.tile([128, TBS], BF16, tag="xsw", bufs=4,
                                       name=f"xsw_{tb}_{dt}")
                    nc.vector.stream_shuffle(xsw[:], raw[:], SWAP_MASK)
                    tmp = scratch.tile([128, TBS], BF16, tag="tmp", bufs=4,
                                       name=f"tmp_{tb}_{dt}")
                    nc.gpsimd.tensor_mul(tmp[:], raw[:], cos_sb[:, tsl])
                    sv = scratch.tile([128, TBS], BF16, tag="sv", bufs=4,
                                      name=f"sv_{tb}_{dt}")
                    nc.vector.tensor_mul(sv[:], xsw[:], sin_sb[:, tsl])
                    qro = scratch.tile([128, TBS], BF16, tag="qro", bufs=4,
                                       name=f"qro_{tb}_{dt}")
                    nc.vector.tensor_add(qro[:], tmp[:], sv[:])
                    dest = qT[:, dt, tsl] if dt < HG else kT[:, tsl]
                    nc.vector.tensor_mul(dest, qro[:], rstdb[:])
                return emit

            for i, dt in enumerate(tail_dts):
                pending_pe.append(make_tail(i, dt))

        flush_pe(len(pending_pe))   # tb3 tails: before any exp is queued

        # ctx^T per head; slots reuse cos/sin/vT space (dead by phase B)
        ctxT = [big.tile([D, S], MM_DT, tag=t, name=f"ctxT_{h}")
                for h, t in enumerate(["cos", "sin", "vT", "ctx3"])]

        # wo loads overlap the first attention blocks ("bigw" frees after
        # the last projection matmul)
        wo_sb = big.tile([128, HG, HID], MM_DT, tag="bigw")
        for ct in range(HG):
            nc.scalar.dma_start(out=wo_sb[:, ct, :],
                                in_=wo[ct * 128:(ct + 1) * 128, :])

        # V transposes ride the first attention block's PE stream, in the
        # (block-0-idle) wo psum banks.
        pending_tp = list(range(KT))

        def flush_tp(k=1):
            for _ in range(min(k, len(pending_tp))):
                kt0 = pending_tp.pop(0)
                tp = psum.tile([128, 128], BF16, tag=["pE", "pF"][kt0 % 2],
                               name=f"tp_{kt0}")
                nc.tensor.transpose(tp[:], vT[:, kt0 * 128:(kt0 + 1) * 128],
                                    ident[:])
                nc.vector.tensor_copy(v_sb[:, kt0, :], tp[:])

        # ---- phase B: attention (qb-major) with Wo folded in ----
        pending_wo = []

        def emit_wo(qb):
            thunks = []
            for tt in range(qb * (TBS // 128), (qb + 1) * (TBS // 128)):
                for hc in range(HID // TBS):
                    def thunk(tt=tt, hc=hc):
                        o_ps = psum.tile([128, TBS], F32,
                                         tag=f"p{'EF'[(tt * 4 + hc) % 2]}",
                                         name=f"o_{tt}_{hc}")
                        for ct in range(HG):
                            nc.tensor.matmul(
                                o_ps[:],
                                ctxT[ct][:, tt * 128:(tt + 1) * 128],
                                wo_sb[:, ct, hc * TBS:(hc + 1) * TBS],
                                start=(ct == 0), stop=(ct == HG - 1))
                        o_sb = outp.tile([128, TBS], F32, tag="osb",
                                         name=f"osb_{tt}_{hc}")
                        nc.scalar.copy(o_sb[:], o_ps[:])
                        nc.sync.dma_start(
                            out=out[tt * 128:(tt + 1) * 128,
                                    hc * TBS:(hc + 1) * TBS],
                            in_=o_sb[:])
                    thunks.append(thunk)
            return thunks

        def flush_wo(k):
            for _ in range(min(k, len(pending_wo))):
                pending_wo.pop(0)()

        # cross-block AV stagger; entries: (kt, e, ctx_ps, blk_i)
        pend = []
        norm_jobs = {}

        def flush_av():
            kt0, e0, c_ps, bi = pend.pop(0)
            nc.tensor.matmul(c_ps[:], v_sb[:, kt0, :], e0[:],
                             start=(kt0 == 0), stop=(kt0 == KT - 1))
            if kt0 == KT - 1 and bi in norm_jobs:
                norm_jobs.pop(bi)()

        for qb in range(QB):
            qsl = slice(qb * TBS, (qb + 1) * TBS)
            for h in range(HG):
                blk_i = qb * HG + h
                ctx_ps = psum.tile([128, TBS], F32,
                                   tag=f"p{'CD'[blk_i % 2]}",
                                   name=f"ctx_{h}_{qb}")
                eacc = scratch.tile([128, TBS], BF16, tag="eacc", bufs=2,
                                    name=f"eacc_{h}_{qb}")
                e_first = None

                for kt in range(KT):
                    s_ps = psum.tile([128, TBS], F32,
                                     tag=f"p{'AB'[kt % 2]}",
                                     name=f"s_{h}_{qb}_{kt}")
                    nc.tensor.matmul(s_ps[:], kT[:, kt * 128:(kt + 1) * 128],
                                     qT[:, h, qsl], start=True, stop=True)
                    e = blk.tile([128, TBS], BF16, tag="blk",
                                 name=f"e_{h}_{qb}_{kt}")
                    nc.scalar.activation(e[:], s_ps[:],
                                         mybir.ActivationFunctionType.Exp,
                                         scale=SCALE)
                    if kt == 0:
                        e_first = e
                    elif kt == 1:
                        nc.vector.tensor_add(eacc[:], e_first[:], e[:])
                    else:
                        nc.vector.tensor_add(eacc[:], eacc[:], e[:])
                    pend.append((kt, e, ctx_ps, blk_i))
                    if len(pend) > STAGGER:
                        flush_av()
                    # block 0: deferred tb3 ssq/tails + V transposes
                    if blk_i == 0 and 3 <= kt <= 10:
                        flush_tp(2)
                    # spread Wo: one output tile (4 matmuls) per slot
                    if qb > 0 and kt % 2 == 1:
                        if (h == 0 and kt >= 5) or h == 1 or \
                           (h == 2 and kt <= 3):
                            flush_wo(1)

                def norm_job(h=h, qb=qb, qsl=qsl, ctx_ps=ctx_ps, eacc=eacc):
                    sum_ps = psum.tile([1, TBS], F32, tag="pG", bufs=1,
                                       name=f"sum_{h}_{qb}")
                    nc.tensor.matmul(sum_ps[:], onesb_col[:], eacc[:],
                                     start=True, stop=True)
                    recip = rows.tile([1, TBS], F32, tag="recip",
                                      name=f"recip_{h}_{qb}")
                    nc.vector.reciprocal_approx_fast(out=recip[:],
                                                     in_=sum_ps[:])
                    recipb = scratch.tile([128, TBS], F32, tag="bcast",
                                          bufs=6, name=f"recipb_{h}_{qb}")
                    nc.gpsimd.partition_broadcast(recipb[:], recip[:])
                    nc.vector.tensor_mul(ctxT[h][:, qsl], ctx_ps[:], recipb[:])
                norm_jobs[blk_i] = norm_job
            pending_wo.extend(emit_wo(qb))

        while pend:
            flush_av()
        for i in sorted(list(norm_jobs)):
            norm_jobs.pop(i)()
        flush_wo(len(pending_wo))

    nc.compile()
    return nc


def _prep_inputs(hidden_states, positions, Wq, Wk, Wv, Wo, q_norm_w, k_norm_w):
    hidden_states = np.asarray(hidden_states, dtype=np.float32)
    positions = np.asarray(positions)
    Wq = np.asarray(Wq, dtype=np.float32)
    Wk = np.asarray(Wk, dtype=np.float32)
    Wv = np.asarray(Wv, dtype=np.float32)
    Wo = np.asarray(Wo, dtype=np.float32)
    q_norm_w = np.asarray(q_norm_w, dtype=np.float32)
    k_norm_w = np.asarray(k_norm_w, dtype=np.float32)

    import ml_dtypes

    # head-dim permutation: pair (j, j+64) -> partitions (2j, 2j+1)
    perm = np.empty(D, dtype=np.int64)
    perm[0::2] = np.arange(HALF)
    perm[1::2] = np.arange(HALF) + HALF

    # permute projection output columns per head
    Wq_p = Wq.reshape(HID, NH, D)[:, :, perm].reshape(HID, NH * D)
    Wk_p = Wk.reshape(HID, NKV, D)[:, :, perm].reshape(HID, NKV * D)
    qnw_p = q_norm_w[perm]
    knw_p = k_norm_w[perm]

    inv_freq = THETA ** (-np.arange(HALF, dtype=np.float32) / HALF)
    in_maps = []
    for c in range(DP * TP):
        b, g = divmod(c, TP)
        freqs = positions[b].astype(np.float32)[:, None] * inv_freq[None, :]
        cos = np.cos(freqs).T.astype(np.float32)      # [64, S]
        sin = np.sin(freqs).T.astype(np.float32)
        # per-pair layout: row 2j/2j+1 both carry cos_j; sin row 2j is
        # -sin_j (even gets -x_odd*sin) and row 2j+1 is +sin_j
        cos2 = np.empty((D, S), dtype=np.float32)
        sin2 = np.empty((D, S), dtype=np.float32)
        cos2[0::2] = cos
        cos2[1::2] = cos
        sin2[0::2] = -sin
        sin2[1::2] = sin
        in_maps.append({
            "xt": np.ascontiguousarray(hidden_states[b].T).astype(PROJ_NP),
            "wq": np.ascontiguousarray(Wq_p[:, g * DQ:(g + 1) * DQ]).astype(PROJ_NP),
            "wk": np.ascontiguousarray(Wk_p[:, g * D:(g + 1) * D]).astype(PROJ_NP),
            "wv": np.ascontiguousarray(Wv[:, g * D:(g + 1) * D]).astype(PROJ_NP),
            "wo": np.ascontiguousarray(Wo[g * DQ:(g + 1) * DQ, :]).astype(MM_NP),
            "cos2": np.ascontiguousarray(cos2),
            "sin2": np.ascontiguousarray(sin2),
            "qnw": np.ascontiguousarray(qnw_p[:, None]),
            "knw": np.ascontiguousarray(knw_p[:, None]),
            "iden": np.eye(128, dtype=ml_dtypes.bfloat16),
            "onesb": np.ones((128, 1), dtype=ml_dtypes.bfloat16),
        })
    return in_maps


def _run(inputs, trace=False):
    skip_w = bool(np.allclose(inputs["q_norm_w"], 1.0)
                  and np.allclose(inputs["k_norm_w"], 1.0))
    key = ("nc", skip_w)
    if key not in _cache:
        _cache[key] = _build(skip_w)
    nc = _cache[key]
    in_maps = _prep_inputs(**inputs)
    res = run_bass_kernel_spmd(nc, in_maps, core_ids=list(range(DP * TP)),
                               trace=trace)
    out = np.zeros((B, S, HID), dtype=np.float32)
    for c in range(DP * TP):
        out[c // TP] += res.results[c]["out"]
    return out, res


def kernel(**inputs):
    out, _ = _run(inputs, trace=False)
    return out
